# revision 13
# baseline (speedup 1.0000x reference)
"""Trainium2 Bass kernel for ragged multi-head self-attention (8 NeuronCores).

Reference computes, per ragged segment (offsets delimit segments of x):
    q,k,v = x@Wq, x@Wk, x@Wv (per-token), softmax(q k^T / sqrt(dh)) v within
    the segment per head, then out = attn@Wo + bo.

Distribution: each segment is cut into 128-query tiles and 128-key chunks.
A shared static "template" (list of (C=kv chunks, S=qtiles) groups) is built
from the actual offsets; all 8 cores run the same graph, the host packs each
core's slice of real work into the template (query-tile level balancing).

Raggedness is handled with exact zero-cost masking: padded key tokens are
zeros (=> scores 0 => exp=1) and a per-token validity column appended to V
zeroes their contribution to both numerator and denominator; the validity
column simultaneously produces the softmax denominator via the same PE
matmul that computes attn^T = V_aug^T @ exp(S^T). No running max is needed
(scores are O(5), exp cannot overflow in fp32).

All matmuls in bf16 with fp32 PSUM accumulation. exp runs on the scalar
engine straight out of PSUM in [128,1536] batches.
"""

import os
from contextlib import ExitStack

import numpy as np

D = 512
H = 8
DH = 64
P = 128
QT = 128  # query tile
CK = 128  # kv chunk
N_CORES = 8
EXPW = 1536  # exp batch width (3 PSUM banks)
QF = 512  # query free-dim per score matmul (one full PSUM bank: PSUM
# start_tensor_calc pending-zero granularity is a 2KB region, so interleaved
# accumulation groups must not share a bank)


# --------------------------------------------------------------------------
# schedule construction (host, from runtime offsets)
# --------------------------------------------------------------------------

def build_schedule(offsets, n_cores=N_CORES):
    offsets = np.asarray(offsets).astype(np.int64)
    lengths = np.diff(offsets)
    assert (lengths > 0).all(), "zero-length segments unsupported"
    segs = [(i, int(offsets[i]), int(L), (int(L) + CK - 1) // CK)
            for i, L in enumerate(lengths)]

    def make_instances(smax_large, smax_small):
        insts = []
        for (si, st, L, C) in segs:
            smax = smax_large if C >= 11 else smax_small
            q0 = 0
            while q0 < C:
                take = min(smax, C - q0)
                insts.append((si, q0, take, C))
                q0 += take
        return insts

    def pack(insts, n_cores):
        insts = sorted(insts, key=lambda t: (-t[3], -t[2]))
        positions = []
        while insts:
            positions.append(insts[:n_cores])
            insts = insts[n_cores:]
        template = [(max(t[3] for t in pos), max(t[2] for t in pos))
                    for pos in positions]
        return positions, template

    best = None
    for sl in (4, 5, 6):
        for ss in (2, 3, 4, 5, 6):
            insts = make_instances(sl, ss)
            if len(insts) > 4 * n_cores:
                continue
            positions, template = pack(insts, n_cores)
            cost = sum(C * S for C, S in template)
            kvcost = sum(C for C, S in template)
            total = cost + 0.2 * kvcost
            if best is None or total < best[0]:
                best = (total, insts, positions, template)
    _, insts, positions, template = best

    core_groups = [[None] * len(template) for _ in range(n_cores)]
    for gpos, pos_insts in enumerate(positions):
        loads = [sum((template[i][0] * template[i][1])
                     if core_groups[c][i] else 0 for i in range(gpos))
                 for c in range(n_cores)]
        order = np.argsort([-t[3] * t[2] for t in pos_insts])
        cores_by_load = list(np.argsort(loads))
        for k, ii in enumerate(order):
            si, q0, nq, C = pos_insts[ii]
            core_groups[cores_by_load[k]][gpos] = (
                si, segs[si][1], segs[si][2], q0, nq)

    NKV = sum(C for C, S in template) * CK
    NQ = sum(S for C, S in template) * QT
    return dict(template=tuple(template), core_groups=core_groups,
                NKV=NKV, NQ=NQ, segs=segs)


def shard_inputs(x, sched, n_cores=N_CORES):
    T, D_ = x.shape
    template = sched["template"]
    NKV, NQ = sched["NKV"], sched["NQ"]
    xqT = np.zeros((n_cores, D_, NQ), np.float32)
    xkvT = np.zeros((n_cores, D_, NKV), np.float32)
    validity = np.zeros((n_cores, NKV), np.float32)
    gather = -np.ones((n_cores, NQ), np.int64)
    for c in range(n_cores):
        kv0 = q0 = 0
        for (C, S), inst in zip(template, sched["core_groups"][c]):
            if inst is not None:
                si, st, L, qlo, nq = inst
                xkvT[c, :, kv0:kv0 + L] = x[st:st + L].T
                validity[c, kv0:kv0 + L] = 1.0
                qs = st + qlo * QT
                qe = min(st + L, qs + nq * QT)
                n = qe - qs
                xqT[c, :, q0:q0 + n] = x[qs:qe].T
                gather[c, q0:q0 + n] = np.arange(qs, qe)
            else:
                validity[c, kv0] = 1.0  # keep denominators > 0
            kv0 += C * CK
            q0 += S * QT
    return xqT, xkvT, validity, gather


# --------------------------------------------------------------------------
# device graph
# --------------------------------------------------------------------------

def build_graph(template, NKV, NQ):
    import concourse.bass as bass
    import concourse.tile as tile
    from concourse import bacc, library_config, mybir

    F32 = mybir.dt.float32
    BF16 = mybir.dt.bfloat16
    EXP = mybir.ActivationFunctionType.Exp

    NKVT = NKV // CK
    nc = bacc.Bacc("TRN2", target_bir_lowering=False, debug=False)

    xqT_d = nc.dram_tensor("xqT", [D, NQ], F32, kind="ExternalInput")
    xkvT_d = nc.dram_tensor("xkvT", [D, NKV], F32, kind="ExternalInput")
    val_d = nc.dram_tensor("validity", [NKV], F32, kind="ExternalInput")
    W_d = {w: nc.dram_tensor(w, [D, D], F32, kind="ExternalInput")
           for w in ("Wq", "Wk", "Wv", "Wo")}
    bo_d = nc.dram_tensor("bo", [D], F32, kind="ExternalInput")
    out_d = nc.dram_tensor("out", [NQ, D], F32, kind="ExternalOutput")

    with ExitStack() as ctx:
        tc = ctx.enter_context(tile.TileContext(nc))
        nc.gpsimd.load_library(library_config.attnmlp)

        # ---- persistent SBUF tensors (one pool, unique tags) ----
        singles = ctx.enter_context(tc.tile_pool(name="singles", bufs=1))

        def single(shape, dtype, name):
            return singles.tile(shape, dtype, name=name, tag=name)

        W_bf = {w: single([P, 4, D], BF16, f"{w}_bf") for w in W_d}
        bo_rep = single([P, D], F32, "bo_rep")
        xqT_bf = single([P, 4, NQ], BF16, "xqT_bf")
        xkvT_bf = single([P, 4, NKV], BF16, "xkvT_bf")
        qT_bf = single([P, 4, NQ], BF16, "qT_bf")
        kT_bf = single([P, 4, NKV], BF16, "kT_bf")
        v_sb = single([P, NKVT, H, 66], BF16, "v_sb")
        val_sb = single([P, NKVT], F32, "val_sb")

        # ---- pools ----
        stage = ctx.enter_context(tc.tile_pool(name="stage", bufs=3))
        psc = ctx.enter_context(tc.tile_pool(name="psc", bufs=2, space="PSUM"))
        pat = ctx.enter_context(tc.tile_pool(name="pat", bufs=1, space="PSUM"))
        ppt = ctx.enter_context(tc.tile_pool(name="ppt", bufs=3))
        prep = ctx.enter_context(tc.tile_pool(name="prep", bufs=2))
        pattn = ctx.enter_context(tc.tile_pool(name="pattn", bufs=2))
        pout = ctx.enter_context(tc.tile_pool(name="pout", bufs=3))

        # ---- load weights + bias ----
        for w in W_d:
            ws = stage.tile([P, 4, D], F32, tag="stage", name=f"{w}_stg")
            nc.sync.dma_start(
                out=ws, in_=W_d[w][:, :].rearrange("(c p) o -> p c o", p=P))
            nc.vector.tensor_copy(out=W_bf[w], in_=ws)
        bo_ap = bo_d[:]
        bo_bcast = bass.AP(tensor=bo_ap.tensor, offset=bo_ap.offset,
                           ap=[[0, P], [1, D]])
        nc.sync.dma_start(out=bo_rep, in_=bo_bcast)
        nc.sync.dma_start(
            out=val_sb, in_=val_d[:].rearrange("(t p) -> p t", p=P))

        # ---- load + cast x ----
        XSTG = 1024
        for (src, dst, N) in ((xqT_d, xqT_bf, NQ), (xkvT_d, xkvT_bf, NKV)):
            for dc in range(4):
                for c0 in range(0, N, XSTG):
                    n = min(XSTG, N - c0)
                    xs = stage.tile([P, XSTG], F32, tag="stage", name="x_stg")
                    nc.sync.dma_start(
                        out=xs[:, :n], in_=src[dc * P:(dc + 1) * P, c0:c0 + n])
                    nc.vector.tensor_copy(
                        out=dst[:, dc, c0:c0 + n], in_=xs[:, :n])

        # ---- q/k projections:  qT[o,t] = sum_d Wq[d,o] x^T[d,t] ----
        for (w, xs_bf, dst, N) in (("Wq", xqT_bf, qT_bf, NQ),
                                   ("Wk", xkvT_bf, kT_bf, NKV)):
            for oc in range(4):
                for c0 in range(0, N, 512):
                    n = min(512, N - c0)
                    ps = psc.tile([P, EXPW], F32, tag="sc", name="proj_ps")
                    for dc in range(4):
                        nc.tensor.matmul(
                            ps[:, :n],
                            lhsT=W_bf[w][:, dc, oc * P:(oc + 1) * P],
                            rhs=xs_bf[:, dc, c0:c0 + n],
                            start=(dc == 0), stop=(dc == 3))
                    nc.vector.tensor_copy(
                        out=dst[:, oc, c0:c0 + n], in_=ps[:, :n])

        # ---- v projection: V[t, o] = sum_d x^T[d, t] Wv[d, o] ----
        for t in range(NKVT):
            ps = psc.tile([P, EXPW], F32, tag="sc", name="v_ps")
            for dc in range(4):
                nc.tensor.matmul(
                    ps[:, :D],
                    lhsT=xkvT_bf[:, dc, t * P:(t + 1) * P],
                    rhs=W_bf["Wv"][:, dc, :],
                    start=(dc == 0), stop=(dc == 3))
            # scatter heads into v_sb[:, t, h, 0:64], cast to bf16
            nc.vector.tensor_copy(
                out=v_sb[:, t, :, 0:DH],
                in_=ps[:, :D].rearrange("p (h d) -> p h d", h=H))
            # validity column per head
            nc.vector.tensor_copy(
                out=v_sb[:, t, :, DH:DH + 1],
                in_=val_sb[:, t:t + 1].to_broadcast([P, H, 1]))

        # ---- attention per template group ----
        kv0 = 0
        q0 = 0
        for gi, (C, S) in enumerate(template):
            nq = S * QT
            # query blocks: widths 512/256/128 (all divide a 2KB PSUM bank).
            # Each qb gets a private 512-wide (bank-aligned) slot in the AV
            # accumulator so interleaved accumulation groups never share a
            # PSUM bank (start=True clears has_written for the whole bank).
            qbs = []
            o = 0
            for wdt in (512, 256, 128):
                while nq - o >= wdt:
                    qbs.append((o, wdt))
                    o += wdt
            nqp = len(qbs) * 512
            attn_bf = pattn.tile([P, 4, nqp], BF16, tag="attn_bf",
                                 name=f"attn_bf_{gi}")
            for h in range(H):
                hp, hr = h % 2, h // 2
                at = pat.tile([DH + 1, nqp], F32, tag="at", name=f"at_{gi}_{h}")
                for qslot, (qo, qn) in enumerate(qbs):
                    # score tiles for this qb, dense-packed (qn | 2048B)
                    per_batch = EXPW // qn
                    c = 0
                    while c < C:
                        nb = min(per_batch, C - c)
                        sc = psc.tile([P, EXPW], F32, tag="sc",
                                      name=f"sc_{gi}_{h}")
                        pt = ppt.tile([P, EXPW], BF16, tag="pt",
                                      name=f"pt_{gi}_{h}")
                        for i in range(nb):
                            nc.tensor.matmul(
                                sc[:, i * qn:(i + 1) * qn],
                                lhsT=kT_bf[hp * DH:(hp + 1) * DH, hr,
                                           (kv0 + c + i) * CK:
                                           (kv0 + c + i + 1) * CK],
                                rhs=qT_bf[hp * DH:(hp + 1) * DH, hr,
                                          q0 + qo:q0 + qo + qn],
                                start=True, stop=True)
                        w = nb * qn
                        nc.scalar.activation(out=pt[:, :w], in_=sc[:, :w],
                                             func=EXP, scale=DH ** -0.5)
                        for i in range(nb):
                            nc.tensor.matmul(
                                at[:, qslot * 512:qslot * 512 + qn],
                                lhsT=v_sb[:, kv0 + c + i, h, 0:DH + 1],
                                rhs=pt[:, i * qn:(i + 1) * qn],
                                start=(c + i == 0), stop=(c + i == C - 1),
                                skip_group_check=True)
                        c += nb
                    # normalize this qb of this head
                    recip = prep.tile([1, 512], F32, tag="recip",
                                      name=f"recip_{gi}_{h}")
                    rep = prep.tile([DH, 512], F32, tag="rep",
                                    name=f"rep_{gi}_{h}")
                    nc.vector.reciprocal(
                        out=recip[:, :qn],
                        in_=at[DH:DH + 1, qslot * 512:qslot * 512 + qn])
                    nc.gpsimd.partition_broadcast(
                        rep[:, :qn], recip[:, :qn], channels=DH)
                    nc.vector.tensor_mul(
                        out=attn_bf[hp * DH:(hp + 1) * DH, hr,
                                    qslot * 512:qslot * 512 + qn],
                        in0=at[0:DH, qslot * 512:qslot * 512 + qn],
                        in1=rep[:, :qn])
            # o-projection + bias per query tile
            for qt in range(S):
                j = next(j for j, (qo, qn) in enumerate(qbs)
                         if qo <= qt * QT < qo + qn)
                pcol = j * 512 + (qt * QT - qbs[j][0])
                po = pat.tile([P, D], F32, tag="at", name=f"po_{gi}_{qt}")
                for dc in range(4):
                    nc.tensor.matmul(
                        po,
                        lhsT=attn_bf[:, dc, pcol:pcol + QT],
                        rhs=W_bf["Wo"][:, dc, :],
                        start=(dc == 0), stop=(dc == 3))
                osb = pout.tile([P, D], F32, tag="osb", name=f"osb_{gi}_{qt}")
                nc.vector.tensor_add(out=osb, in0=po, in1=bo_rep)
                nc.sync.dma_start(
                    out=out_d[q0 + qt * QT:q0 + (qt + 1) * QT, :], in_=osb)
            kv0 += C
            q0 += nq
    nc.compile()  # bacc lowering (strips tile pseudo-insts for walrus)
    return nc


# --------------------------------------------------------------------------
# entry point
# --------------------------------------------------------------------------

_GRAPH_CACHE = {}


def kernel(x, Wq, Wk, Wv, Wo, bo, offsets):
    from concourse.bass_utils import run_bass_kernel_spmd

    x = np.ascontiguousarray(np.asarray(x, np.float32))
    offsets_np = np.asarray(offsets)
    sched = build_schedule(offsets_np)
    key = (tuple(sched["template"]), sched["NKV"], sched["NQ"])
    if key not in _GRAPH_CACHE:
        _GRAPH_CACHE[key] = build_graph(*key)
    nc = _GRAPH_CACHE[key]

    xqT, xkvT, validity, gather = shard_inputs(x, sched)
    Wq = np.asarray(Wq, np.float32)
    Wk = np.asarray(Wk, np.float32)
    Wv = np.asarray(Wv, np.float32)
    Wo = np.asarray(Wo, np.float32)
    bo = np.asarray(bo, np.float32)
    in_maps = [
        dict(xqT=xqT[c], xkvT=xkvT[c], validity=validity[c],
             Wq=Wq, Wk=Wk, Wv=Wv, Wo=Wo, bo=bo)
        for c in range(N_CORES)
    ]
    import time as _time
    _t0 = _time.monotonic()
    res = run_bass_kernel_spmd(nc, in_maps, core_ids=list(range(N_CORES)),
                               trace=bool(os.environ.get("KERNEL_TRACE")))
    kernel.last_run_s = _time.monotonic() - _t0
    kernel.last_results = res

    T = x.shape[0]
    out = np.zeros((T, D), np.float32)
    for c in range(N_CORES):
        m = gather[c] >= 0
        out[gather[c][m]] = res.results[c]["out"][m]
    return out


# revision 24
# speedup vs baseline: 20426.3141x; 20426.3141x over previous
"""Trainium2 Bass kernel for ragged multi-head self-attention (8 NeuronCores).

Reference computes, per ragged segment (offsets delimit segments of x):
    q,k,v = x@Wq, x@Wk, x@Wv (per-token), softmax(q k^T / sqrt(dh)) v within
    the segment per head, then out = attn@Wo + bo.

Distribution: each segment is cut into 128-query tiles and 128-key chunks.
A shared static "template" (list of (C=kv chunks, S=qtiles) groups) is built
from the actual offsets; all 8 cores run the same graph, the host packs each
core's slice of real work into the template (query-tile level balancing).

Raggedness is handled with exact zero-cost masking: padded key tokens are
zeros (=> scores 0 => exp=1) and a per-token validity column appended to V
zeroes their contribution to both numerator and denominator; the validity
column simultaneously produces the softmax denominator via the same PE
matmul that computes attn^T = V_aug^T @ exp(S^T). No running max is needed
(scores are O(5), exp cannot overflow in fp32).

All matmuls in bf16 with fp32 PSUM accumulation. exp runs on the scalar
engine straight out of PSUM in [128,1536] batches.
"""

import os
from contextlib import ExitStack

import numpy as np

D = 512
H = 8
DH = 64
P = 128
QT = 128  # query tile
CK = 128  # kv chunk
N_CORES = 8
EXPW = 1024  # exp batch width (PSUM banks x 512)
SC_BUFS = 3   # score buffers
AT_BUFS = 1   # AV accumulator buffers
CAST_ENGINE = "vector"  # engine for SBUF->SBUF casts (vector|gpsimd)
QF = 512  # query free-dim per score matmul (one full PSUM bank: PSUM
# start_tensor_calc pending-zero granularity is a 2KB region, so interleaved
# accumulation groups must not share a bank)


# --------------------------------------------------------------------------
# schedule construction (host, from runtime offsets)
# --------------------------------------------------------------------------

def build_schedule(offsets, n_cores=N_CORES):
    offsets = np.asarray(offsets).astype(np.int64)
    lengths = np.diff(offsets)
    assert (lengths > 0).all(), "zero-length segments unsupported"
    segs = [(i, int(offsets[i]), int(L), (int(L) + CK - 1) // CK)
            for i, L in enumerate(lengths)]

    def make_instances(smax_large, smax_small):
        insts = []
        for (si, st, L, C) in segs:
            smax = smax_large if C >= 11 else smax_small
            q0 = 0
            while q0 < C:
                take = min(smax, C - q0)
                insts.append((si, q0, take, C))
                q0 += take
        return insts

    def pack(insts, n_cores):
        insts = sorted(insts, key=lambda t: (-t[3], -t[2]))
        positions = []
        while insts:
            positions.append(insts[:n_cores])
            insts = insts[n_cores:]
        template = [(max(t[3] for t in pos), max(t[2] for t in pos))
                    for pos in positions]
        return positions, template

    best = None
    for sl in (4, 5, 6):
        for ss in (2, 3, 4, 5, 6):
            insts = make_instances(sl, ss)
            if len(insts) > 4 * n_cores:
                continue
            positions, template = pack(insts, n_cores)
            units = sum(C * S for C, S in template)
            kvtiles = sum(C for C, S in template)
            act = 1.10 * units
            pe = 0.85 * units + 1.55 * kvtiles
            total = max(act, pe) + 0.25 * min(act, pe)
            if best is None or total < best[0]:
                best = (total, insts, positions, template)
    if best is None:  # many tiny segments: no split option fit the cap
        insts = make_instances(6, 6)
        positions, template = pack(insts, n_cores)
        units = sum(C * S for C, S in template)
        best = (units, insts, positions, template)
    _, insts, positions, template = best

    core_groups = [[None] * len(template) for _ in range(n_cores)]
    for gpos, pos_insts in enumerate(positions):
        loads = [sum((template[i][0] * template[i][1])
                     if core_groups[c][i] else 0 for i in range(gpos))
                 for c in range(n_cores)]
        order = np.argsort([-t[3] * t[2] for t in pos_insts])
        cores_by_load = list(np.argsort(loads))
        for k, ii in enumerate(order):
            si, q0, nq, C = pos_insts[ii]
            core_groups[cores_by_load[k]][gpos] = (
                si, segs[si][1], segs[si][2], q0, nq)

    NKV = sum(C for C, S in template) * CK
    NQ = sum(S for C, S in template) * QT
    return dict(template=tuple(template), core_groups=core_groups,
                NKV=NKV, NQ=NQ, segs=segs)


def shard_inputs(x, sched, n_cores=N_CORES):
    T, D_ = x.shape
    template = sched["template"]
    NKV, NQ = sched["NKV"], sched["NQ"]
    xqT = np.zeros((n_cores, D_, NQ), np.float32)
    xkvT = np.zeros((n_cores, D_, NKV), np.float32)
    validity = np.zeros((n_cores, NKV), np.float32)
    gather = -np.ones((n_cores, NQ), np.int64)
    for c in range(n_cores):
        kv0 = q0 = 0
        for (C, S), inst in zip(template, sched["core_groups"][c]):
            if inst is not None:
                si, st, L, qlo, nq = inst
                xkvT[c, :, kv0:kv0 + L] = x[st:st + L].T
                validity[c, kv0:kv0 + L] = 1.0
                qs = st + qlo * QT
                qe = min(st + L, qs + nq * QT)
                n = qe - qs
                xqT[c, :, q0:q0 + n] = x[qs:qe].T
                gather[c, q0:q0 + n] = np.arange(qs, qe)
            else:
                validity[c, kv0] = 1.0  # keep denominators > 0
            kv0 += C * CK
            q0 += S * QT
    return xqT, xkvT, validity, gather


# --------------------------------------------------------------------------
# device graph
# --------------------------------------------------------------------------

def build_graph(template, NKV, NQ):
    import concourse.bass as bass
    import concourse.tile as tile
    from concourse import bacc, library_config, mybir

    F32 = mybir.dt.float32
    BF16 = mybir.dt.bfloat16
    EXP = mybir.ActivationFunctionType.Exp

    NKVT = NKV // CK
    nc = bacc.Bacc("TRN2", target_bir_lowering=False, debug=False)

    xqT_d = nc.dram_tensor("xqT", [D, NQ], F32, kind="ExternalInput")
    xkvT_d = nc.dram_tensor("xkvT", [D, NKV], F32, kind="ExternalInput")
    val_d = nc.dram_tensor("validity", [NKV], F32, kind="ExternalInput")
    W_d = {w: nc.dram_tensor(w, [D, D], F32, kind="ExternalInput")
           for w in ("Wq", "Wk", "Wv", "Wo")}
    bo_d = nc.dram_tensor("bo", [D], F32, kind="ExternalInput")
    out_d = nc.dram_tensor("out", [NQ, D], F32, kind="ExternalOutput")

    with ExitStack() as ctx:
        tc = ctx.enter_context(tile.TileContext(nc))
        nc.gpsimd.load_library(library_config.attnmlp)

        # ---- persistent SBUF tensors (one pool, unique tags) ----
        singles = ctx.enter_context(tc.tile_pool(name="singles", bufs=1))

        def single(shape, dtype, name):
            return singles.tile(shape, dtype, name=name, tag=name)

        W_bf = {w: single([P, 4, D], BF16, f"{w}_bf") for w in W_d}
        bo_rep = single([P, D], F32, "bo_rep")
        xqT_bf = single([P, 4, NQ], BF16, "xqT_bf")
        xkvT_bf = single([P, 4, NKV], BF16, "xkvT_bf")
        qT_bf = single([P, 4, NQ], BF16, "qT_bf")
        kT_bf = single([P, 4, NKV], BF16, "kT_bf")
        v_sb = single([P, NKVT, H, 66], BF16, "v_sb")
        val_sb = single([P, NKVT], F32, "val_sb")

        # ---- pools ----
        stage = ctx.enter_context(tc.tile_pool(name="stage", bufs=3))
        psc = ctx.enter_context(tc.tile_pool(name="psc", bufs=SC_BUFS, space="PSUM"))
        pat = ctx.enter_context(tc.tile_pool(name="pat", bufs=AT_BUFS, space="PSUM"))
        ppt = ctx.enter_context(tc.tile_pool(name="ppt", bufs=3))
        praw = ctx.enter_context(tc.tile_pool(name="praw", bufs=3))
        prep = ctx.enter_context(tc.tile_pool(name="prep", bufs=4))
        pattn = ctx.enter_context(tc.tile_pool(name="pattn", bufs=2))
        pout = ctx.enter_context(tc.tile_pool(name="pout", bufs=3))

        # ---- load weights + bias (k/v first; per-chunk casts) ----
        for w in ("Wk", "Wv", "Wq", "Wo"):
            for dc in range(4):
                ws = stage.tile([P, D], F32, tag="wstage", name=f"{w}_stg")
                nc.sync.dma_start(
                    out=ws, in_=W_d[w][dc * P:(dc + 1) * P, :])
                getattr(nc, CAST_ENGINE).tensor_copy(
                    out=W_bf[w][:, dc, :], in_=ws)
        bo_ap = bo_d[:]
        bo_bcast = bass.AP(tensor=bo_ap.tensor, offset=bo_ap.offset,
                           ap=[[0, P], [1, D]])
        nc.sync.dma_start(out=bo_rep, in_=bo_bcast)
        nc.sync.dma_start(
            out=val_sb, in_=val_d[:].rearrange("(t p) -> p t", p=P))

        # ---- load + cast x (group-0's kv data first so PE starts early) --
        XSTG = 1024

        def load_ranges(srcs):
            for (srcd, dst, lo, hi) in srcs:
                o = lo
                while o < hi:
                    n = min(XSTG, hi - o)
                    for dc in range(4):
                        xs = stage.tile([P, XSTG], F32, tag="stage",
                                        name="x_stg")
                        nc.sync.dma_start(
                            out=xs[:, :n],
                            in_=srcd[dc * P:(dc + 1) * P, o:o + n])
                        getattr(nc, CAST_ENGINE).tensor_copy(
                            out=dst[:, dc, o:o + n], in_=xs[:, :n])
                    o += n

        kvlo = 0
        qlo = 0
        for (C, S) in template:
            load_ranges([(xkvT_d, xkvT_bf, kvlo * CK, (kvlo + C) * CK),
                         (xqT_d, xqT_bf, qlo * QT, (qlo + S) * QT)])
            kvlo += C
            qlo += S

        # ---- per-group projection emitters (interleaved into the
        # previous group's attention batches so PE fills exp-wait gaps) ----
        def col_blocks(lo, hi, w=512):
            o = lo
            while o < hi:
                yield o, min(w, hi - o)
                o += w

        def proj_items(gi):
            C, S = template[gi]
            kv_lo = sum(c for c, s in template[:gi]) * CK
            kv_hi = kv_lo + C * CK
            q_lo = sum(s for c, s in template[:gi]) * QT
            q_hi = q_lo + S * QT
            items = []

            def kproj(c0, n):
                def emit():
                    ps = psc.tile([P, EXPW], F32, tag="sc", name="kp_ps")
                    for dc in range(4):
                        for oc in range(4):
                            nc.tensor.matmul(
                                ps[:, oc * 128:oc * 128 + n] if False else
                                ps[:, :n],
                                lhsT=W_bf["Wk"][:, dc, 0:P],
                                rhs=xkvT_bf[:, dc, c0:c0 + n],
                                start=(dc == 0), stop=(dc == 3))
                return emit
            # k-proj: per (oc, block): psum accumulate over dc, then copy
            for oc in range(4):
                for c0, n in col_blocks(kv_lo, kv_hi):
                    def emit(oc=oc, c0=c0, n=n):
                        ps = psc.tile([P, EXPW], F32, tag="sc", name="kp_ps")
                        for dc in range(4):
                            nc.tensor.matmul(
                                ps[:, :n],
                                lhsT=W_bf["Wk"][:, dc, oc * P:(oc + 1) * P],
                                rhs=xkvT_bf[:, dc, c0:c0 + n],
                                start=(dc == 0), stop=(dc == 3))
                        nc.vector.tensor_copy(
                            out=kT_bf[:, oc, c0:c0 + n], in_=ps[:, :n])
                    items.append(emit)
            # v-proj per kv tile + validity columns for the group
            for t in range(kv_lo // CK, kv_hi // CK):
                def emit(t=t):
                    ps = psc.tile([P, EXPW], F32, tag="sc", name="v_ps")
                    for dc in range(4):
                        nc.tensor.matmul(
                            ps[:, :D],
                            lhsT=xkvT_bf[:, dc, t * P:(t + 1) * P],
                            rhs=W_bf["Wv"][:, dc, :],
                            start=(dc == 0), stop=(dc == 3))
                    nc.vector.tensor_copy(
                        out=v_sb[:, t, :, 0:DH],
                        in_=ps[:, :D].rearrange("p (h d) -> p h d", h=H))
                items.append(emit)

            def emit_val():
                t0, t1 = kv_lo // CK, kv_hi // CK
                vap = val_sb[:, t0:t1]
                rep8 = bass.AP(tensor=vap.tensor, offset=vap.offset,
                               ap=list(vap.ap[:2]) + [[0, H]])
                nc.vector.tensor_copy(out=v_sb[:, t0:t1, :, DH], in_=rep8)
            items.append(emit_val)
            # q-proj
            for oc in range(4):
                for c0, n in col_blocks(q_lo, q_hi):
                    def emit(oc=oc, c0=c0, n=n):
                        ps = psc.tile([P, EXPW], F32, tag="sc", name="qp_ps")
                        for dc in range(4):
                            nc.tensor.matmul(
                                ps[:, :n],
                                lhsT=W_bf["Wq"][:, dc, oc * P:(oc + 1) * P],
                                rhs=xqT_bf[:, dc, c0:c0 + n],
                                start=(dc == 0), stop=(dc == 3))
                        nc.vector.tensor_copy(
                            out=qT_bf[:, oc, c0:c0 + n], in_=ps[:, :n])
                    items.append(emit)
            return items

        # ---- attention per template group ----
        kv0 = 0
        q0 = 0
        pending = []
        for it in proj_items(0):
            it()
        for gi, (C, S) in enumerate(template):
            nq = S * QT
            pending.extend(proj_items(gi + 1) if gi + 1 < len(template) else [])
            # query blocks: widths 512/256/128 (all divide a 2KB PSUM bank).
            # Each qb gets a private 512-wide (bank-aligned) slot in the AV
            # accumulator so interleaved accumulation groups never share a
            # PSUM bank (start=True clears has_written for the whole bank).
            qbs = []
            o = 0
            for wdt in (512, 256, 128):
                while nq - o >= wdt:
                    qbs.append((o, wdt))
                    o += wdt
            nqp = len(qbs) * 512
            attn_bf = pattn.tile([P, 4, nqp], BF16, tag="attn_bf",
                                 name=f"attn_bf_{gi}")
            # Build the full batch list for this group (all heads/qslots),
            # then emit software-pipelined: scores_b, exp_b, AV_{b-1} --
            # so PE computes batch b's scores while ACT runs exp of b-1,
            # instead of stalling for exp_b before AV_b.
            batches = []  # (h, qslot, qo, qn, c0, nb)
            for h in range(H):
                for qslot, (qo, qn) in enumerate(qbs):
                    per_batch = EXPW // qn
                    c = 0
                    while c < C:
                        nb = min(per_batch, C - c)
                        batches.append((h, qslot, qo, qn, c, nb))
                        c += nb
            at_tiles = {}

            def emit_scores(b):
                h, qslot, qo, qn, c0, nb = b
                hp, hr = h % 2, h // 2
                sc = psc.tile([P, EXPW], F32, tag="sc", name=f"sc_{gi}_{h}")
                pt = ppt.tile([P, EXPW], BF16, tag="pt", name=f"pt_{gi}_{h}")
                for i in range(nb):
                    nc.tensor.matmul(
                        sc[:, i * qn:(i + 1) * qn],
                        lhsT=kT_bf[hp * DH:(hp + 1) * DH, hr,
                                   (kv0 + c0 + i) * CK:(kv0 + c0 + i + 1) * CK],
                        rhs=qT_bf[hp * DH:(hp + 1) * DH, hr,
                                  q0 + qo:q0 + qo + qn],
                        start=True, stop=True)
                w = nb * qn
                nc.scalar.activation(out=pt[:, :w], in_=sc[:, :w],
                                     func=EXP, scale=DH ** -0.5)
                return pt

            def emit_av(b, pt):
                h, qslot, qo, qn, c0, nb = b
                hp, hr = h % 2, h // 2
                if h not in at_tiles:
                    at_tiles[h] = pat.tile([DH + 1, nqp], F32, tag="at",
                                           name=f"at_{gi}_{h}")
                at = at_tiles[h]
                for i in range(nb):
                    nc.tensor.matmul(
                        at[:, qslot * 512:qslot * 512 + qn],
                        lhsT=v_sb[:, kv0 + c0 + i, h, 0:DH + 1],
                        rhs=pt[:, i * qn:(i + 1) * qn],
                        start=(c0 + i == 0), stop=(c0 + i == C - 1),
                        skip_group_check=True)
                if c0 + nb == C:
                    # last batch of this (h, qslot): drain + normalize
                    raw = praw.tile([DH + 1, 512], F32, tag="raw",
                                    name=f"raw_{gi}_{h}_{qslot}")
                    nc.vector.tensor_copy(
                        out=raw[:, :qn],
                        in_=at[:, qslot * 512:qslot * 512 + qn])
                    if qslot == len(qbs) - 1:
                        del at_tiles[h]
                    recip = prep.tile([1, 512], F32, tag="recip",
                                      name=f"recip_{gi}_{h}")
                    rep = prep.tile([DH, 512], F32, tag="rep",
                                    name=f"rep_{gi}_{h}")
                    nc.vector.reciprocal(
                        out=recip[:, :qn], in_=raw[DH:DH + 1, :qn])
                    nc.gpsimd.partition_broadcast(
                        rep[:, :qn], recip[:, :qn], channels=DH)
                    nc.vector.tensor_mul(
                        out=attn_bf[hp * DH:(hp + 1) * DH, hr,
                                    qslot * 512:qslot * 512 + qn],
                        in0=raw[0:DH, :qn], in1=rep[:, :qn])

            pend_av = None
            for b in batches:
                pt = emit_scores(b)
                if pending:
                    pending.pop(0)()
                if pend_av is not None:
                    emit_av(*pend_av)
                pend_av = (b, pt)
            emit_av(*pend_av)
            while pending:
                pending.pop(0)()
            # o-projection + bias per query tile (deferred into the next
            # group's batch stream when possible)
            def oproj_item(qt, gi=gi, qbs=qbs, attn_bf=attn_bf, q0=q0):
                def emit():
                    j = next(j for j, (qo, qn) in enumerate(qbs)
                             if qo <= qt * QT < qo + qn)
                    pcol = j * 512 + (qt * QT - qbs[j][0])
                    po = psc.tile([P, EXPW], F32, tag="sc",
                                  name=f"po_{gi}_{qt}")
                    for dc in range(4):
                        nc.tensor.matmul(
                            po[:, :D],
                            lhsT=attn_bf[:, dc, pcol:pcol + QT],
                            rhs=W_bf["Wo"][:, dc, :],
                            start=(dc == 0), stop=(dc == 3))
                    osb = pout.tile([P, D], F32, tag="osb",
                                    name=f"osb_{gi}_{qt}")
                    nc.vector.tensor_add(out=osb[:], in0=po[:, :D],
                                         in1=bo_rep)
                    nc.sync.dma_start(
                        out=out_d[q0 + qt * QT:q0 + (qt + 1) * QT, :],
                        in_=osb)
                return emit

            opitems = [oproj_item(qt) for qt in range(S)]
            if gi + 1 < len(template):
                pending.extend(opitems)
            else:
                for it in opitems:
                    it()
            kv0 += C
            q0 += nq
    nc.compile()  # bacc lowering (strips tile pseudo-insts for walrus)
    return nc


# --------------------------------------------------------------------------
# entry point
# --------------------------------------------------------------------------

_GRAPH_CACHE = {}


def kernel(x, Wq, Wk, Wv, Wo, bo, offsets):
    from concourse.bass_utils import run_bass_kernel_spmd

    x = np.ascontiguousarray(np.asarray(x, np.float32))
    offsets_np = np.asarray(offsets)
    sched = build_schedule(offsets_np)
    key = (tuple(sched["template"]), sched["NKV"], sched["NQ"])
    if key not in _GRAPH_CACHE:
        _GRAPH_CACHE[key] = build_graph(*key)
    nc = _GRAPH_CACHE[key]

    xqT, xkvT, validity, gather = shard_inputs(x, sched)
    Wq = np.asarray(Wq, np.float32)
    Wk = np.asarray(Wk, np.float32)
    Wv = np.asarray(Wv, np.float32)
    Wo = np.asarray(Wo, np.float32)
    bo = np.asarray(bo, np.float32)
    in_maps = [
        dict(xqT=xqT[c], xkvT=xkvT[c], validity=validity[c],
             Wq=Wq, Wk=Wk, Wv=Wv, Wo=Wo, bo=bo)
        for c in range(N_CORES)
    ]
    import time as _time
    _t0 = _time.monotonic()
    res = run_bass_kernel_spmd(nc, in_maps, core_ids=list(range(N_CORES)),
                               trace=bool(os.environ.get("KERNEL_TRACE")))
    kernel.last_run_s = _time.monotonic() - _t0
    kernel.last_results = res

    T = x.shape[0]
    out = np.zeros((T, D), np.float32)
    for c in range(N_CORES):
        m = gather[c] >= 0
        out[gather[c][m]] = res.results[c]["out"][m]
    return out
